# revision 9
# baseline (speedup 1.0000x reference)
"""MultiHeadAttention forward on 8 Trainium2 NeuronCores (Bass/Tile).

Problem (hardcoded): B=2, S=2048, D=1024, H=16, DK=64, causal mask.

Sharding: data-parallel over batch (2) x tensor-parallel over heads
(4 heads per core).  core = 4*b + g handles batch b, heads [4g, 4g+4).
Each core computes its Q/K/V projections (256 output dims), attention
for its 4 heads, and a partial O-projection (contraction over its 256
context dims).  The host sums the 4 partial O outputs per batch and
adds bo (no on-device collectives).

Per-core kernel layout (feature-major "transposed" orientation):
  host sends qT/kT/vT = x[b].T  [1024, 2048]  (layout choice at shard time)
  QT[do, s]  = wqT.T @ qT  (+bq)     [256, 2048]   do-tile t in {0,1} = head pair
  KT[do, s]  = wkT.T @ kT  (+bk)     [256, 2048]
  V[s, dv]   = vT.T @ wvT  (+bv)     [2048, 260]  4x(64 + ones-col)
  S^T[k, q]  = KT_h.T @ QT_h         per (pair, q-chunk 512, k-tile 128),
               two K=64 matmuls packed on PE row-groups (tile_position)
  P = exp(S^T/8) (ScalarE, no max-subtraction: |scores/8| < ~5 for this
               data regime), causal masking applied multiplicatively
  ctx^T[dv, q], den[q] = [V_h | ones].T @ P   (M=65, denominator rides along)
  ctx_norm = ctx^T * (1/den)  (gpsimd partition-broadcast of recip row)
  out[s, :] += ctx_pair.T @ woT  (seq-major partial O-projection)

All matmuls run as float32r (full-rate fp32 mode, free dim >= 256).
"""

import os
import sys

sys.path.insert(0, "/opt/trn_rl_repo")

import numpy as np

B, S, D, H = 2, 2048, 1024, 16
DK = D // H          # 64
NCORES = 8
G = 4                # heads per core
DG = G * DK          # 256 output dims per core
SC = 512             # seq chunk
NCH = S // SC        # 4
KI = D // 128        # 8 contraction chunks
NPAIR = 2            # head pairs per core
NST = SC // 128      # 4 seq 128-tiles per chunk

_programs = {}
LAST_RESULT = None


def _build_program(causal: bool, reps: int = 1):
    import concourse.bass as bass
    import concourse.tile as tile
    import concourse.mybir as mybir
    from concourse import bacc
    from contextlib import ExitStack

    F32 = mybir.dt.float32
    F32R = mybir.dt.float32r
    AF = mybir.ActivationFunctionType

    nc = bacc.Bacc()
    qT_d = nc.dram_tensor("qT", [D, S], F32, kind="ExternalInput")
    kT_d = nc.dram_tensor("kT", [D, S], F32, kind="ExternalInput")
    vT_d = nc.dram_tensor("vT", [D, S], F32, kind="ExternalInput")
    wq_d = nc.dram_tensor("wqT", [D, DG], F32, kind="ExternalInput")
    wk_d = nc.dram_tensor("wkT", [D, DG], F32, kind="ExternalInput")
    wv_d = nc.dram_tensor("wvT", [D, DG], F32, kind="ExternalInput")
    wo_d = nc.dram_tensor("woT", [DG, D], F32, kind="ExternalInput")
    bq_d = nc.dram_tensor("bq2", [128, NPAIR], F32, kind="ExternalInput")
    bk_d = nc.dram_tensor("bk2", [128, NPAIR], F32, kind="ExternalInput")
    bv_d = nc.dram_tensor("bvb", [128, DG], F32, kind="ExternalInput")
    tri_d = nc.dram_tensor("tri", [128, 128], F32, kind="ExternalInput")
    ones_d = nc.dram_tensor("ones4", [128, G], F32, kind="ExternalInput")
    out_d = nc.dram_tensor("out", [S, D], F32, kind="ExternalOutput")

    with ExitStack() as ctx:
        tc = ctx.enter_context(tile.TileContext(nc))
        cpool = ctx.enter_context(tc.tile_pool(name="const", bufs=1))
        inpool = ctx.enter_context(tc.tile_pool(name="ins", bufs=KI + 2))
        qtpool = ctx.enter_context(tc.tile_pool(name="qt", bufs=1))
        ktpool = ctx.enter_context(tc.tile_pool(name="kt", bufs=1))
        vpool = ctx.enter_context(tc.tile_pool(name="v", bufs=1))
        cxpool = ctx.enter_context(tc.tile_pool(name="cx", bufs=1))
        prpool = ctx.enter_context(tc.tile_pool(name="probs", bufs=3))
        rcpool = ctx.enter_context(tc.tile_pool(name="recip", bufs=1))
        bcpool = ctx.enter_context(tc.tile_pool(name="bc", bufs=2))
        tmpool = ctx.enter_context(tc.tile_pool(name="tmp", bufs=1))
        outpool = ctx.enter_context(tc.tile_pool(name="osb", bufs=2))
        psA = ctx.enter_context(tc.tile_pool(name="psA", bufs=2, space="PSUM"))
        psC = ctx.enter_context(tc.tile_pool(name="psC", bufs=4, space="PSUM"))

        # ---- constants
        wq_sb = cpool.tile([128, KI * DG], F32, tag="wq")
        wk_sb = cpool.tile([128, KI * DG], F32, tag="wk")
        wv_sb = cpool.tile([128, KI * DG], F32, tag="wv")
        wo_sb = cpool.tile([128, 2 * D], F32, tag="wo")
        nc.sync.dma_start(
            wq_sb[:].rearrange("p (a m) -> p a m", a=KI).bitcast(F32R),
            wq_d[:].rearrange("(a p) m -> p a m", p=128).bitcast(F32R),
        )
        nc.sync.dma_start(
            wk_sb[:].rearrange("p (a m) -> p a m", a=KI).bitcast(F32R),
            wk_d[:].rearrange("(a p) m -> p a m", p=128).bitcast(F32R),
        )
        nc.sync.dma_start(
            wv_sb[:].rearrange("p (a m) -> p a m", a=KI).bitcast(F32R),
            wv_d[:].rearrange("(a p) m -> p a m", p=128).bitcast(F32R),
        )
        nc.sync.dma_start(
            wo_sb[:].rearrange("p (a m) -> p a m", a=2).bitcast(F32R),
            wo_d[:].rearrange("(a p) m -> p a m", p=128).bitcast(F32R),
        )
        bq_sb = cpool.tile([128, NPAIR], F32, tag="bq")
        bk_sb = cpool.tile([128, NPAIR], F32, tag="bk")
        bv_sb = cpool.tile([128, DG], F32, tag="bv")
        nc.sync.dma_start(bq_sb[:], bq_d[:])
        nc.sync.dma_start(bk_sb[:], bk_d[:])
        nc.sync.dma_start(bv_sb[:], bv_d[:])
        tri_sb = cpool.tile([128, 128], F32, tag="tri")
        nc.sync.dma_start(tri_sb[:], tri_d[:])

        QT = {}
        KT = {}
        V = {}
        CX = {}

        if reps > 1:
            # benchmarking variant: repeat the whole body on-device so
            # steady-state per-iteration time can be extracted from wall
            # clock (no NTFF profiling available in this environment)
            ctx.enter_context(tc.For_i(0, reps, 1))

        for c in range(NCH):
            cs = slice(c * SC, (c + 1) * SC)
            # ---- stream inputs for this seq chunk, one [128,512] tile per
            # d_in block (fine-grained for pipelining across chunks)
            qin, kin, vin = [], [], []
            for tiles, tag, t_d in (
                (qin, "qin", qT_d),
                (kin, "kin", kT_d),
                (vin, "vin", vT_d),
            ):
                for ki in range(KI):
                    t_sb = inpool.tile([128, SC], F32, tag=tag)
                    nc.sync.dma_start(
                        t_sb[:].bitcast(F32R),
                        t_d[ki * 128 : (ki + 1) * 128, cs].bitcast(F32R),
                    )
                    tiles.append(t_sb)

            # ---- Q/K projections: QT[t][c] = [128 do, 512 s]
            for w_sb, b_sb, in_sb, dst in (
                (wq_sb, bq_sb, qin, QT),
                (wk_sb, bk_sb, kin, KT),
            ):
                for t in range(NPAIR):
                    ps = psA.tile([128, SC], F32, tag="mm")
                    for ki in range(KI):
                        nc.tensor.matmul(
                            ps[:],
                            w_sb[:, ki * DG + t * 128 : ki * DG + (t + 1) * 128].bitcast(F32R),
                            in_sb[ki][:].bitcast(F32R),
                            start=(ki == 0),
                            stop=(ki == KI - 1),
                        )
                    tl = (qtpool if dst is QT else ktpool).tile(
                        [128, SC], F32, tag=f"{'q' if dst is QT else 'k'}{t}{c}"
                    )
                    nc.vector.tensor_scalar_add(tl[:].bitcast(F32R), ps[:], b_sb[:, t : t + 1])
                    dst[(t, c)] = tl

            # ---- V projection, seq-major with ones columns: V[j] = [128 s, 4*65]
            for st in range(NST):
                j = c * NST + st
                ps = psA.tile([128, DG], F32, tag="mm")
                for ki in range(KI):
                    nc.tensor.matmul(
                        ps[:],
                        vin[ki][:, st * 128 : (st + 1) * 128].bitcast(F32R),
                        wv_sb[:, ki * DG : (ki + 1) * DG].bitcast(F32R),
                        start=(ki == 0),
                        stop=(ki == KI - 1),
                    )
                vt = vpool.tile([128, G * 65], F32, tag=f"v{j}")
                nc.sync.dma_start(
                    vt[:].rearrange("p (h x) -> p h x", x=65)[:, :, 64:65].bitcast(F32R),
                    ones_d[:].rearrange("p (h x) -> p h x", x=1).bitcast(F32R),
                )
                nc.vector.tensor_add(
                    vt[:].rearrange("p (h x) -> p h x", x=65)[:, :, 0:64].bitcast(F32R),
                    ps[:].rearrange("p (h x) -> p h x", x=64),
                    bv_sb[:].rearrange("p (h x) -> p h x", x=64),
                )
                V[j] = vt

            # ---- attention for q-chunk c, per head pair
            for p in range(NPAIR):
                njt = NST * (c + 1) if causal else NST * NCH
                ctx0 = psC.tile([65, SC], F32, tag="ctx")
                ctx1 = psC.tile([65, SC], F32, tag="ctx")
                h0, h1 = 2 * p, 2 * p + 1
                pending = None

                def emit_av(j, probs, first, last):
                    nc.tensor.matmul(
                        ctx0[:],
                        V[j][:, 65 * h0 : 65 * h0 + 65].bitcast(F32R),
                        probs[:, 0:SC].bitcast(F32R),
                        start=first,
                        stop=last,
                    )
                    nc.tensor.matmul(
                        ctx1[:],
                        V[j][:, 65 * h1 : 65 * h1 + 65].bitcast(F32R),
                        probs[:, SC : 2 * SC].bitcast(F32R),
                        start=first,
                        stop=last,
                    )

                for j in range(njt):
                    jc, jt = divmod(j, NST)
                    scp = psA.tile([128, 2 * SC], F32, tag="mm")
                    nc.tensor.matmul(
                        scp[:, 0:SC],
                        KT[(p, jc)][0:64, jt * 128 : (jt + 1) * 128].bitcast(F32R),
                        QT[(p, c)][0:64, :].bitcast(F32R),
                        start=True,
                        stop=True,
                        tile_position=(0, 0),
                    )
                    nc.tensor.matmul(
                        scp[:, SC : 2 * SC],
                        KT[(p, jc)][64:128, jt * 128 : (jt + 1) * 128].bitcast(F32R),
                        QT[(p, c)][64:128, :].bitcast(F32R),
                        start=True,
                        stop=True,
                        tile_position=(64, 0),
                    )
                    probs = prpool.tile([128, 2 * SC], F32, tag="probs")
                    nc.scalar.activation(probs[:].bitcast(F32R), scp[:], AF.Exp, scale=0.125)
                    if causal and j >= NST * c:
                        m = j - NST * c
                        if m > 0:
                            pv = probs[:].rearrange("p (h x) -> p h x", x=SC)[
                                :, :, 0 : 128 * m
                            ]
                            nc.vector.tensor_scalar_mul(pv.bitcast(F32R), pv, 0.0)
                        for off in (0, SC):
                            lo = off + 128 * m
                            nc.vector.tensor_mul(
                                probs[:, lo : lo + 128].bitcast(F32R),
                                probs[:, lo : lo + 128],
                                tri_sb[:],
                            )
                    if pending is not None:
                        emit_av(*pending)
                    pending = (j, probs, j == 0, j == njt - 1)
                emit_av(*pending)

                # ---- normalize: recip of denominator rows, broadcast, scale.
                # Stage the recip rows in bc's own row 64 (overwritten by the
                # broadcast right after the DMA moves them to partition 0).
                bc = bcpool.tile([128, 2 * SC], F32, tag="bc")
                nc.vector.reciprocal(bc[64:65, 0:SC], ctx0[64:65, :])
                nc.vector.reciprocal(bc[64:65, SC : 2 * SC], ctx1[64:65, :])
                recip = rcpool.tile([1, 2 * SC], F32, tag="recip")
                nc.sync.dma_start(recip[0:1, :], bc[64:65, :])
                nc.gpsimd.partition_broadcast(bc[:], recip[:])
                cx = cxpool.tile([128, SC], F32, tag=f"cx{p}{c}")
                nc.vector.tensor_mul(cx[0:64, :].bitcast(F32R), ctx0[0:64, :], bc[0:64, 0:SC])
                tmp = tmpool.tile([64, SC], F32, tag="tmp")
                nc.vector.tensor_mul(tmp[:].bitcast(F32R), ctx1[0:64, :], bc[0:64, SC : 2 * SC])
                nc.sync.dma_start(cx[64:128, :].bitcast(F32R), tmp[:].bitcast(F32R))
                CX[(p, c)] = cx

            # ---- partial O-projection for this chunk (seq-major)
            for st in range(NST):
                osb = outpool.tile([128, D], F32, tag="osb")
                for n in range(2):
                    ps = psA.tile([128, SC], F32, tag="mm")
                    for p in range(NPAIR):
                        nc.tensor.matmul(
                            ps[:],
                            CX[(p, c)][:, st * 128 : (st + 1) * 128].bitcast(F32R),
                            wo_sb[:, p * D + n * SC : p * D + (n + 1) * SC].bitcast(F32R),
                            start=(p == 0),
                            stop=(p == NPAIR - 1),
                        )
                    nc.vector.tensor_copy(osb[:, n * SC : (n + 1) * SC], ps[:])
                r0 = (c * NST + st) * 128
                nc.sync.dma_start(out_d[r0 : r0 + 128, :], osb[:])

    nc.finalize()
    return nc


def get_program(causal: bool):
    if causal not in _programs:
        _programs[causal] = _build_program(causal)
    return _programs[causal]


def _make_core_inputs(query, key, value, wq, bq, wk, bk, wv, bv, wo):
    f32 = np.float32
    tri = np.triu(np.ones((128, 128), f32))
    in_maps = []
    for core in range(NCORES):
        b, g = divmod(core, G)
        sl = slice(g * DG, (g + 1) * DG)
        in_maps.append(
            {
                "qT": np.ascontiguousarray(query[b].T, f32),
                "kT": np.ascontiguousarray(key[b].T, f32),
                "vT": np.ascontiguousarray(value[b].T, f32),
                "wqT": np.ascontiguousarray(wq[sl, :].T, f32),
                "wkT": np.ascontiguousarray(wk[sl, :].T, f32),
                "wvT": np.ascontiguousarray(wv[sl, :].T, f32),
                "woT": np.ascontiguousarray(wo[:, sl].T, f32),
                "bq2": np.ascontiguousarray(bq[sl].reshape(NPAIR, 128).T, f32),
                "bk2": np.ascontiguousarray(bk[sl].reshape(NPAIR, 128).T, f32),
                "bvb": np.ascontiguousarray(
                    np.broadcast_to(bv[sl], (128, DG)), f32
                ),
                "tri": tri,
                "ones4": np.ones((128, G), f32),
            }
        )
    return in_maps


def _numpy_fallback(query, key, value, mask, wq, bq, wk, bk, wv, bv, wo, bo):
    out = np.empty((B, S, D), np.float32)
    for b in range(B):
        Q = (query[b] @ wq.T + bq).reshape(S, H, DK).transpose(1, 0, 2)
        K = (key[b] @ wk.T + bk).reshape(S, H, DK).transpose(1, 0, 2)
        Vv = (value[b] @ wv.T + bv).reshape(S, H, DK).transpose(1, 0, 2)
        sc = np.einsum("hqd,hkd->hqk", Q, K) / np.sqrt(np.float32(DK))
        sc = np.where(mask[b][None] == 0, -np.inf, sc)
        sc = sc - sc.max(axis=-1, keepdims=True)
        e = np.exp(sc)
        attn = e / e.sum(axis=-1, keepdims=True)
        ctx = np.einsum("hqk,hkd->hqd", attn, Vv)
        out[b] = ctx.transpose(1, 0, 2).reshape(S, D) @ wo.T + bo
    return out


def kernel(query, key, value, mask, wq, bq, wk, bk, wv, bv, wo, bo):
    global LAST_RESULT
    query = np.asarray(query, np.float32)
    key = np.asarray(key, np.float32)
    value = np.asarray(value, np.float32)
    mask = np.asarray(mask)
    wq, bq = np.asarray(wq, np.float32), np.asarray(bq, np.float32)
    wk, bk = np.asarray(wk, np.float32), np.asarray(bk, np.float32)
    wv, bv = np.asarray(wv, np.float32), np.asarray(bv, np.float32)
    wo, bo = np.asarray(wo, np.float32), np.asarray(bo, np.float32)

    tril = np.tril(np.ones((S, S), mask.dtype))
    if all((mask[b] == tril).all() for b in range(B)):
        causal = True
    elif (mask == 1).all():
        causal = False
    else:
        return _numpy_fallback(
            query, key, value, mask, wq, bq, wk, bk, wv, bv, wo, bo
        )

    from concourse.bass_utils import run_bass_kernel_spmd

    nc = get_program(causal)
    in_maps = _make_core_inputs(query, key, value, wq, bq, wk, bk, wv, bv, wo)
    trace = bool(int(os.environ.get("MHA_TRACE", "0")))
    res = run_bass_kernel_spmd(nc, in_maps, list(range(NCORES)), trace=trace)
    LAST_RESULT = res

    out = np.zeros((B, S, D), np.float32)
    for core in range(NCORES):
        b = core // G
        out[b] += res.results[core]["out"]
    out += bo[None, None, :]
    return out
